# revision 10
# baseline (speedup 1.0000x reference)
"""Causal self-attention (B=2, T=2048, D=2048, H=16, HD=128) on 8 TRN2 cores.

The per-execution cost on this stack is dominated by operand staging
(~0.4-0.6 ms per MB per core), so the kernel is designed to minimize staged
bytes: every tensor is sharded 8 ways host-side and the full activations are
reassembled on device with collectives.

  - x is sharded over tokens (2 MB/core, fp16) and AllGathered on device to
    the full [D, NT] layout each core needs for its heads.
  - W q/k/v rows are sharded over heads (3 MB/core), W_o is sharded over its
    input (head) dim (1 MB/core): each core computes an o-proj partial for
    ALL tokens and a ReduceScatter(add) sums partials while sharding tokens,
    writing the [512, D] fp16 output slice directly.
  - cos/sin are staged once as a [128, T] tile (0.5 MB), duplicated/negated
    on device; the causal mask triangle and the all-ones tile are built on
    device (memset/iota-free: 32 KB triangle staged as input).

All tensors and matmuls are fp16 (same PE throughput as bf16, 3 extra
mantissa bits); PSUM accumulation and softmax statistics stay fp32. Softmax
uses exp(s*scale - 6): the RMS norm bounds |s*scale| <= sqrt(128), so the
shift makes fp16 overflow impossible; the shift cancels in the p/sum(p)
normalization.

Total staged operands: ~6.5 MB/core inputs + 2 MB/core output buffer vs
~33.5 MB/core for the replicated baseline.
"""

import numpy as np

B, T, D = 2, 2048, 2048
H, HD = 16, 128
N_CORES = 8
HPC = H // N_CORES          # heads per core
NT = B * T                  # 4096 tokens, b-major
TS = NT // N_CORES          # 512-token shard per core
DC = D // 128               # 16 contraction chunks
NTT = NT // 512             # 8 token blocks
KT_PER_B = T // 128         # 16 k-tiles per batch row

_CACHE = {}


def _build(scale: float, reps: int = 1):
    import concourse.bacc as bacc
    import concourse.mybir as mybir
    import concourse.tile as tile

    f32 = mybir.dt.float32
    MM = mybir.dt.float16
    EPS = float(np.finfo(np.float32).eps)
    SHIFT = -6.0

    nc = bacc.Bacc("TRN2", target_bir_lowering=False, debug=False,
                   num_devices=N_CORES)

    # declaration order = staging order: x first so the AllGather can fire
    # while the weights are still staging
    xTs_d = nc.dram_tensor("xTs", [D, TS], MM, kind="ExternalInput")
    wqk_d = nc.dram_tensor("wqk", [D, 4 * 128], MM, kind="ExternalInput")
    wv_d = nc.dram_tensor("wv", [D, HPC * HD], MM, kind="ExternalInput")
    cs_d = nc.dram_tensor("cs", [128, T], MM, kind="ExternalInput")
    tri_d = nc.dram_tensor("tri", [128, 128], MM, kind="ExternalInput")
    wo_d = nc.dram_tensor("wo", [HPC * HD, D], MM, kind="ExternalInput")
    y_d = nc.dram_tensor("y", [TS, D], MM, kind="ExternalOutput")

    Sq = mybir.ActivationFunctionType.Square
    Sqrt = mybir.ActivationFunctionType.Sqrt
    Exp = mybir.ActivationFunctionType.Exp
    Copy = mybir.ActivationFunctionType.Copy

    with tile.TileContext(nc) as tc:
        with tc.tile_pool(name="dram", bufs=1, space="DRAM") as dram, \
             tc.tile_pool(name="res", bufs=1) as res:
            ag_in = dram.tile([D, TS], MM, tag="ag_in", name="ag_in")
            op_d = dram.tile([N_CORES, TS, D], MM, tag="op_d", name="op_d")
            rs_out = dram.tile([TS, D], MM, tag="rs_out", name="rs_out")

            # Residents: rotated q/k, v in [token, hd] layout, weights,
            # rotary tables, attention output (transposed), small constants.
            qk_sb = res.tile([128, 4 * NT], MM, tag="qk")
            v_sb = res.tile([128, (NT // 128) * (HPC * HD)], MM, tag="v")
            ynT_sb = res.tile([128, HPC * NT], MM, tag="ynT")
            wqk_sb = res.tile([128, DC * 512], MM, tag="wqk")
            wv_sb = res.tile([128, DC * HPC * HD], MM, tag="wv")
            wo_sb = res.tile([128, HPC * D], MM, tag="wo")
            c_sb = res.tile([128, T], MM, tag="c_sb")
            s_sb = res.tile([128, T], MM, tag="s_sb")
            tri_sb = res.tile([128, 128], MM, tag="tri")
            ones_sb = res.tile([128, 128], MM, tag="ones")
            eps_sb = res.tile([128, 1], f32, tag="eps")
            shift_sb = res.tile([128, 1], f32, tag="shift")
            nc.vector.memset(eps_sb[:], EPS)
            nc.vector.memset(shift_sb[:], SHIFT)
            nc.vector.memset(ones_sb[:], 1.0)

            for _rep in range(reps):
                # ---- AllGather x: [D, TS] per core -> [NTT, D, TS] ----
                # (collectives cannot read/write IO tensors; bounce via DRAM.
                #  Shared DRAM allows only one writer -> one ag_out per rep)
                ag_out = dram.tile([NTT, D, TS], MM, tag=f"ag_out{_rep}",
                                   name=f"ag_out{_rep}", addr_space="Shared")
                nc.sync.dma_start(out=ag_in[:, :], in_=xTs_d[:, :])
                nc.gpsimd.collective_compute(
                    "AllGather",
                    mybir.AluOpType.bypass,
                    replica_groups=[list(range(N_CORES))],
                    ins=[ag_in.opt()],
                    outs=[ag_out.opt()],
                )

                # weight/table loads overlap the AllGather
                nc.sync.dma_start(
                    out=wqk_sb[:].rearrange("p (c f) -> p c f", f=512),
                    in_=wqk_d[:, :].rearrange("(c p) f -> p c f", p=128))
                nc.sync.dma_start(
                    out=wv_sb[:].rearrange("p (c f) -> p c f", f=256),
                    in_=wv_d[:, :].rearrange("(c p) f -> p c f", p=128))
                nc.sync.dma_start(
                    out=wo_sb[:].rearrange("p (c f) -> p c f", f=D),
                    in_=wo_d[:, :].rearrange("(c p) f -> p c f", p=128))
                nc.sync.dma_start(out=c_sb[0:64, :], in_=cs_d[0:64, :])
                nc.sync.dma_start(out=c_sb[64:128, :], in_=cs_d[0:64, :])
                nc.sync.dma_start(out=s_sb[64:128, :], in_=cs_d[64:128, :])
                ssrc = res.tile([64, T], MM, tag="ssrc")
                nc.sync.dma_start(out=ssrc[:], in_=cs_d[64:128, :])
                nc.scalar.activation(s_sb[0:64, :], ssrc[:], Copy, scale=-1.0)
                nc.sync.dma_start(out=tri_sb[:], in_=tri_d[:, :])

                # ---------------- Phase 1: QKV + RMS norm + rotary ----------------
                with tc.tile_pool(name="xs", bufs=3) as xs, \
                     tc.tile_pool(name="st", bufs=3) as st, \
                     tc.tile_pool(name="ps1", bufs=2, space="PSUM") as ps1:
                    for n in range(NTT):
                        xblk = xs.tile([128, DC * 512], MM, tag="xblk")
                        for cg in range(4):
                            nc.sync.dma_start(
                                out=xblk[:, cg * 4 * 512:(cg + 1) * 4 * 512]
                                    .rearrange("p (c f) -> p c f", f=512),
                                in_=ag_out[n, cg * 512:(cg + 1) * 512, :]
                                    .rearrange("(c p) f -> p c f", p=128))
                        # v projection: [token, hd] layout
                        for c4 in range(4):
                            vps = ps1.tile([128, HPC * HD], f32, tag="vps")
                            for dc in range(DC):
                                nc.tensor.matmul(
                                    vps[:],
                                    xblk[:, dc * 512 + c4 * 128: dc * 512 + (c4 + 1) * 128],
                                    wv_sb[:, dc * 256:(dc + 1) * 256],
                                    start=(dc == 0), stop=(dc == DC - 1))
                            tcg = n * 4 + c4
                            nc.vector.tensor_copy(v_sb[:, tcg * 256:(tcg + 1) * 256], vps[:])
                        # q/k projection + rms + rotary, m-chunks q0,q1,k0,k1
                        for m in range(4):
                            qps = ps1.tile([128, 512], f32, tag="qps")
                            for dc in range(DC):
                                nc.tensor.matmul(
                                    qps[:],
                                    wqk_sb[:, dc * 512 + m * 128: dc * 512 + (m + 1) * 128],
                                    xblk[:, dc * 512:(dc + 1) * 512],
                                    start=(dc == 0), stop=(dc == DC - 1))
                            sq = st.tile([128, 512], MM, tag="sq")
                            nc.scalar.activation(sq[:], qps[:], Sq)
                            ssq = ps1.tile([128, 512], f32, tag="ssq")
                            nc.tensor.matmul(ssq[:], ones_sb[:], sq[:], start=True, stop=True)
                            rms = st.tile([128, 512], f32, tag="rms")
                            nc.scalar.activation(rms[:], ssq[:], Sqrt, bias=eps_sb[:], scale=1.0 / HD)
                            r = st.tile([128, 512], f32, tag="r")
                            nc.vector.reciprocal(r[:], rms[:])
                            qn = st.tile([128, 512], MM, tag="qn")
                            nc.vector.tensor_mul(qn[:], qps[:], r[:])
                            # rotary: y = qn*C + swap(qn)*S  with S = [-sin; sin]
                            tsw = st.tile([128, 512], MM, tag="tsw")
                            tt = (n % 4) * 512
                            ctile = c_sb[:, tt:tt + 512]
                            stile = s_sb[:, tt:tt + 512]
                            nc.vector.tensor_mul(tsw[0:64, :], qn[64:128, :], stile[64:128, :])
                            nc.vector.tensor_mul(tsw[64:128, :], qn[0:64, :], stile[0:64, :])
                            dst = qk_sb[:, m * NT + n * 512: m * NT + (n + 1) * 512]
                            nc.vector.tensor_mul(dst, qn[:], ctile)
                            nc.vector.tensor_add(dst, dst, tsw[:])

                # ---------------- Phase 2: attention ----------------
                with tc.tile_pool(name="p2", bufs=4) as p2, \
                     tc.tile_pool(name="p2b", bufs=2) as p2b, \
                     tc.tile_pool(name="pss", bufs=2, space="PSUM") as pss, \
                     tc.tile_pool(name="psd", bufs=2, space="PSUM") as psd, \
                     tc.tile_pool(name="psy", bufs=2, space="PSUM") as psy:
                    for h in range(HPC):
                        qoff = h * NT
                        koff = (2 + h) * NT
                        for b in range(B):
                            for qj in range(4):
                                yps = psy.tile([128, 512], f32, tag="yps")
                                dps = psd.tile([128, 512], f32, tag="dps")
                                nkt = 4 * qj + 4
                                qbase = qoff + b * T + qj * 512
                                for kb in range(nkt):
                                    # diagonal blocks: only q-columns >= 128*mi live
                                    lo = max(0, (kb - 4 * qj) * 128)
                                    sps = pss.tile([128, 512], f32, tag="sps")
                                    nc.tensor.matmul(
                                        sps[:, lo:],
                                        qk_sb[:, koff + b * T + kb * 128: koff + b * T + (kb + 1) * 128],
                                        qk_sb[:, qbase + lo: qbase + 512],
                                        start=True, stop=True)
                                    e = p2.tile([128, 512], MM, tag="e")
                                    nc.scalar.activation(e[:, lo:], sps[:, lo:], Exp,
                                                         bias=shift_sb[:], scale=scale)
                                    if kb >= 4 * qj:
                                        # triangle mask on the diagonal 128 cols
                                        nc.vector.tensor_mul(
                                            e[:, lo:lo + 128], e[:, lo:lo + 128], tri_sb[:])
                                    nc.tensor.matmul(dps[:, lo:], ones_sb[:], e[:, lo:],
                                                     start=(kb == 0), stop=(kb == nkt - 1))
                                    tcg = b * KT_PER_B + kb
                                    nc.tensor.matmul(
                                        yps[:, lo:],
                                        v_sb[:, tcg * 256 + h * 128: tcg * 256 + (h + 1) * 128],
                                        e[:, lo:],
                                        start=(kb == 0), stop=(kb == nkt - 1))
                                rcp = p2b.tile([128, 512], f32, tag="rcp")
                                nc.vector.reciprocal(rcp[:], dps[:])
                                dst = ynT_sb[:, h * NT + b * T + qj * 512:
                                             h * NT + b * T + (qj + 1) * 512]
                                nc.vector.tensor_mul(dst, yps[:], rcp[:])

                # ---------------- Phase 3: o-proj partials + ReduceScatter ----------------
                with tc.tile_pool(name="p3", bufs=4) as p3, \
                     tc.tile_pool(name="ps3", bufs=4, space="PSUM") as ps3:
                    for tcg in range(NT // 128):
                        g, rr = tcg // 4, tcg % 4
                        for on in range(4):
                            ops = ps3.tile([128, 512], f32, tag="ops")
                            for h in range(HPC):
                                nc.tensor.matmul(
                                    ops[:],
                                    ynT_sb[:, h * NT + tcg * 128:
                                           h * NT + (tcg + 1) * 128],
                                    wo_sb[:, h * D + on * 512:h * D + (on + 1) * 512],
                                    start=(h == 0), stop=(h == HPC - 1))
                            pp = p3.tile([128, 512], MM, tag="pp")
                            nc.scalar.activation(pp[:], ops[:], Copy)
                            nc.sync.dma_start(
                                out=op_d[g, rr * 128:(rr + 1) * 128,
                                         on * 512:(on + 1) * 512],
                                in_=pp[:])
                    nc.gpsimd.collective_compute(
                        "ReduceScatter",
                        mybir.AluOpType.add,
                        replica_groups=[list(range(N_CORES))],
                        ins=[op_d.opt()],
                        outs=[rs_out.opt()],
                    )
                    nc.sync.dma_start(out=y_d[:, :], in_=rs_out.opt())

    nc.compile()
    return nc


def _prep_inputs(x, W, cos, sin):
    import concourse.mybir as mybir
    fp = mybir.dt.np(mybir.dt.float16)

    xT = np.ascontiguousarray(x.reshape(NT, D).T).astype(fp)
    cs = np.concatenate([cos.T, sin.T], 0).astype(fp)   # [128, T]
    tri = (np.arange(128)[:, None] <= np.arange(128)[None, :]).astype(fp)
    in_maps = []
    for c in range(N_CORES):
        r0 = c * HPC * HD
        wqk = np.ascontiguousarray(
            np.concatenate([W[0][r0:r0 + 256], W[1][r0:r0 + 256]], 0).T).astype(fp)
        wv = np.ascontiguousarray(W[2][r0:r0 + 256].T).astype(fp)
        wo = np.ascontiguousarray(W[3][:, r0:r0 + 256].T).astype(fp)
        in_maps.append({
            "xTs": np.ascontiguousarray(xT[:, c * TS:(c + 1) * TS]),
            "wqk": wqk, "wv": wv, "wo": wo, "cs": cs, "tri": tri,
        })
    return in_maps


def kernel(x, W, cos, sin, scale):
    from concourse.bass_utils import run_bass_kernel_spmd

    x = np.asarray(x, dtype=np.float32)
    W = np.asarray(W, dtype=np.float32)
    cos = np.asarray(cos, dtype=np.float32)
    sin = np.asarray(sin, dtype=np.float32)
    sc = float(np.asarray(scale))

    if sc not in _CACHE:
        _CACHE[sc] = _build(sc)
    nc = _CACHE[sc]

    in_maps = _prep_inputs(x, W, cos, sin)
    out = run_bass_kernel_spmd(nc, in_maps, core_ids=list(range(N_CORES)))
    y = np.concatenate([out.results[c]["y"] for c in range(N_CORES)], axis=0)
    return y.astype(np.float32).reshape(B, T, D)
